# revision 6
# baseline (speedup 1.0000x reference)
"""Trainium2 Bass kernel for nn_Attention_62938450756123.

Reference computation (per batch b):
    oe[s, h] = out_e[s, b, 0:512] + out_e[s, b, 512:1024]      # bidirectional sum
    od[t, h] = out_d[t, b, :]
    S[s, t]  = sum_h oe[s, h] * od[t, h]
    p[s, t]  = exp(S[s, t])                                     # naive, no max-sub
    ctx[t,h] = (sum_s p[s, t] * oe[s, h]) / (sum_s p[s, t])
    out[t, b, h] = ctx[t, h]

Sharding: data-parallel over batch (bs=16) across 8 NeuronCores, 2 batches
per core, no collectives.

Per-core dataflow:
  - GPSIMD (SWDGE) cast-loads f32->bf16: out_e halves + out_d tiles.  The
    very first out_d chunk instead goes through sync HWDGE as raw f32 +
    Scalar Copy-cast: HWDGE starts transferring immediately while SWDGE
    takes ~10us to ramp, so the PE gets transpose work right as the HAM
    warmup ends.
  - VectorE sums the out_e halves -> oe tiles bf16 [s128, h512] (mm2 rhs).
  - h-major layouts for mm1 are built ON TensorE: for each 128x128 block,
    psum[h, s'] = sum_s x[s, h] * (SCALE * I[s, s'])  (normal matmul,
    scaled identity moving).  Four h-chunks pack into one PSUM bank; one
    VectorE copy casts the bank to fp8e4m3 SBUF:
    oeT_i [128p, 4hc, 128s], odT_chunk [128p, 4hc, 512t], h = hc*128 + p,
    values pre-scaled by SCALE=32 to sit in fp8's normal range.
    (DMA-xbar transposes are NOT used: Tile serializes them against every
    other DMA - HW-deadlock workaround - which ping-pongs the load stream.)
  - mm1 runs in fp8 with perf_mode=DoubleRow (2 fp8 weights/PE cell):
    psum_S[s128, t512] accumulates over 2 k-tiles of [128p x 2ko] = 256.
    ScalarE exp applies scale=1/SCALE^2 to undo the identity pre-scaling:
    P = exp(S_psum / 1024) in bf16.  Softmax output error stays ~1e-3.
  - d8 = DSCALE*(P - 1) in fp8: the p = 1 + d decomposition quantizes ~5x
    better than bf16 quantizes p itself (|d| <= ~0.06*DSCALE sits in
    fp8e4m3's normal range).  d8 conversions ALTERNATE between DVE and
    GPSIMD - the DVE is the load-phase bottleneck (adds+casts+d8).
  - mm2: the ctx accumulation is split into two half-width PSUM tiles per
    t-tile (h 0:256 | 256:512, padded to 260 cols for 4B-aligned moving
    strides) and the oe8 moving operand carries a ones column at index
    256, so the softmax DENOMINATOR accumulates as column 256 of the same
    matmuls (no separate psD/constD instructions disrupting the psC
    stream; the old psD layout measured +59ns per pair):
      psC_a[t128, 260] = K=1 bcast of [DSCALE*colsum[0:256] | DSCALE*SL|1s]
                       + sum_pairs d8.T @ [oe8[:,0:256] | 1...]   (fp8 DR)
      psC_b likewise for h 256:512; rc = recip(psC_a[:, 256]); DSCALE
    cancels in psC * rc.  The a-group completes before the b-group so the
    reciprocal overlaps the b stream.
  - normalize on VectorE (reciprocal + 2 half tensor_scalars), store via
    Sync HWDGE.
  - ~8us dummy-matmul warmup un-throttles the HAM PE clock gate; the
    first loads are issued BEFORE it so their transfers stream meanwhile.
  - Cross-batch software pipeline: batch 0's prep runs flat (DMA-paced,
    all mm1 inside), then batch 1's ENTIRE prep (loads, transposes,
    mm1+exp+d8, colsum) is fed into batch 0's mm2 tail one bundle per
    t-tile boundary - between accumulation groups, never inside them -
    so the tail's idle Scalar/GpSimd/DMA absorb batch 1's load phase
    while the PE stream stays dense.  mm1 ops are WOVEN with per-k
    e-transpose halves inside each bundle: the Scalar exp chain (700ns
    per pair vs 430ns of matmuls) otherwise paces the psS pipeline.

Buffers are allocated per-s-tile (separate Tile objects) so dependency
tracking stays precise (Tile tracks deps at tile granularity).
"""

import ml_dtypes
import numpy as np

import concourse.bass as bass
import concourse.tile as tile
from concourse import bacc, mybir
from concourse.bass_utils import run_bass_kernel_spmd

SL, TL, BS, H = 2048, 2048, 16, 512
NCORES = 8
BPC = BS // NCORES  # batches per core

F32 = mybir.dt.float32
BF16 = mybir.dt.bfloat16
FP8 = mybir.dt.float8e4

NS = SL // 128        # 16 s-tiles
NH = H // 128         # 4 h-chunks
TCHUNK = 512          # t-chunk (one PSUM bank of f32)
NTC = TL // TCHUNK    # 4 t-chunks
TPC = TCHUNK // 128   # 4 t-tiles per chunk
SCALE = 32.0          # fp8 pre-scale (folded into the transpose identity)
DSCALE = 16.0         # fp8 pre-scale for d = p - 1 (|d| <~ 0.06 -> ~1)
HH = H // 2           # mm2 half-width (256)
HP = HH + 4           # half-width + den column + 3 pad cols (260)


def build():
    nc = bacc.Bacc("TRN2", target_bir_lowering=False, debug=False,
                   num_devices=NCORES)
    out_e = nc.dram_tensor("out_e", [SL, BPC, 2 * H], F32,
                           kind="ExternalInput").ap()
    out_d = nc.dram_tensor("out_d", [TL, BPC, H], F32,
                           kind="ExternalInput").ap()
    ident = nc.dram_tensor("ident", [128, 128], BF16,
                           kind="ExternalInput").ap()
    out = nc.dram_tensor("out", [TL, BPC, H], F32,
                         kind="ExternalOutput").ap()

    exp = mybir.ActivationFunctionType.Exp
    dr = mybir.MatmulPerfMode.DoubleRow

    with tile.TileContext(nc) as tc:
        with (
            tc.tile_pool(name="consts", bufs=1) as consts,
            tc.tile_pool(name="stage_e", bufs=6) as stage_e_pool,
            tc.tile_pool(name="stage_d", bufs=4) as stage_d_pool,
            tc.tile_pool(name="oenat", bufs=2 * NS) as oenat_pool,
            tc.tile_pool(name="oet", bufs=2 * NS) as oet_pool,
            tc.tile_pool(name="odt", bufs=2 * NTC) as odt_pool,
            tc.tile_pool(name="pbuf", bufs=8) as p_pool,
            tc.tile_pool(name="d8buf", bufs=2 * NS) as d8_pool,
            tc.tile_pool(name="oe8buf", bufs=2 * NS) as oe8_pool,
            tc.tile_pool(name="osb", bufs=3) as osb_pool,
            tc.tile_pool(name="small", bufs=4) as small_pool,
            tc.tile_pool(name="psS", bufs=2, space="PSUM") as psS_pool,
            tc.tile_pool(name="psC", bufs=4, space="PSUM") as psC_pool,
            tc.tile_pool(name="ptr", bufs=2, space="PSUM") as ptr_pool,
        ):
            ones = consts.tile([128, 1], BF16, tag="ones")
            nc.vector.memset(ones, 1.0)
            onesK1 = consts.tile([1, 128], BF16, tag="onesK1")
            nc.vector.memset(onesK1, 1.0)
            idt = consts.tile([128, 128], BF16, tag="idt")
            nc.sync.dma_start(idt, ident)

            def transpose_tiles(src, dst):
                """src [128, NH*128] bf16 -> dst [128, NH, 128] fp8 with
                dst[p, c, j] = SCALE * src[j, c*128 + p], via NH identity
                matmuls packed into one PSUM bank + one DVE copy-cast."""
                pt = ptr_pool.tile([128, NH * 128], F32, tag="ptr")
                for c in range(NH):
                    nc.tensor.matmul(pt[:, c * 128:(c + 1) * 128],
                                     src[:, c * 128:(c + 1) * 128], idt,
                                     start=True, stop=True)
                nc.vector.tensor_copy(dst, pt)

            class BatchState:
                def __init__(self, b):
                    self.b = b
                    self.oe_tiles = [None] * NS   # [128, H] bf16 (colsum)
                    self.oe8_pairs = [None] * (NS // 2)  # [128, 2, 2, HP]
                    #     fp8 mm2 rhs, ones+den cols at HH:HP
                    self.oeT_tiles = [None] * NS  # [128, NH, 128] fp8
                    self.odT_chunks = [None] * NTC  # [128, NH, TCHUNK] fp8
                    self.d8_pairs = {tci: [None] * (NS // 2)
                                     for tci in range(NTC)}
                    self.sd = [None] * NTC
                    self.st = [None] * (NS // 2)
                    self.cs2 = None

            def dma_d(S, ci, split=False):
                # one t-chunk (4 t-tiles) per merged SWDGE cast-load;
                # split=True: 4 per-k sync-HWDGE f32 loads + Scalar
                # Copy-casts so the first transposes start ~10us earlier
                # (HWDGE transfers start immediately; SWDGE ramps slowly,
                # and the Scalar engine is idle until the first exp)
                if split:
                    S.sd[ci] = [
                        stage_d_pool.tile([128, H], BF16, tag="sd0",
                                          bufs=TPC, name=f"sd0_{k}")
                        for k in range(TPC)]
                    fks = []
                    for k in range(TPC):
                        r0 = ci * TCHUNK + k * 128
                        f32k = stage_d_pool.tile([128, H], F32, tag="sdf",
                                                 bufs=2, name=f"sdf_{k}")
                        nc.sync.dma_start(f32k, out_d[r0:r0 + 128, S.b, :])
                        fks.append(f32k)
                    for k in range(TPC):
                        nc.scalar.activation(
                            S.sd[ci][k], fks[k],
                            mybir.ActivationFunctionType.Copy)
                    return
                sd = stage_d_pool.tile([128, TPC, H], BF16, tag="sd",
                                       name=f"sd_{S.b}_{ci}")
                S.sd[ci] = sd
                src = out_d[ci * TCHUNK:(ci + 1) * TCHUNK, S.b, :]
                nc.gpsimd.dma_start(
                    sd, src.rearrange("(k p) h -> p k h", p=128))

            def tr_d_split(S, ci):
                odc = odt_pool.tile([128, NH, TCHUNK], FP8, tag="odT",
                                    name=f"odT_{S.b}_{ci}")
                S.odT_chunks[ci] = odc
                for k in range(TPC):
                    transpose_tiles(S.sd[ci][k],
                                    odc[:, :, k * 128:(k + 1) * 128])

            def tr_d(S, ci):
                odc = odt_pool.tile([128, NH, TCHUNK], FP8, tag="odT",
                                    name=f"odT_{S.b}_{ci}")
                S.odT_chunks[ci] = odc
                for k in range(TPC):
                    transpose_tiles(S.sd[ci][:, k, :],
                                    odc[:, :, k * 128:(k + 1) * 128])

            def dma_e(S, j):
                # two s-tiles (both halves) per merged SWDGE cast-load
                st = stage_e_pool.tile([128, 2, 2 * H], BF16, tag="st",
                                       name=f"st_{S.b}_{j}")
                S.st[j] = st
                src = out_e[j * 256:(j + 1) * 256, S.b, :]
                nc.gpsimd.dma_start(
                    st, src.rearrange("(k p) h -> p k h", p=128))

            def dma_e_split(S, j):
                # e-tile via 2 per-k Scalar-HWDGE f32 loads (no cast: the
                # DVE adds read f32 directly).  Used for the first tile so
                # its transfer streams while SWDGE is still ramping.
                stF = [stage_e_pool.tile([128, 2 * H], F32, tag="stF",
                                         bufs=2, name=f"stF_{k}")
                       for k in range(2)]
                S.st[j] = stF
                for k in range(2):
                    r0 = j * 256 + k * 128
                    nc.scalar.dma_start(stF[k], out_e[r0:r0 + 128, S.b, :])

            def tr_e_k(S, j, k):
                st = S.st[j]
                if isinstance(st, list):
                    sta, stb = st[k][:, 0:H], st[k][:, H:2 * H]
                else:
                    sta, stb = st[:, k, 0:H], st[:, k, H:2 * H]
                if k == 0:
                    oe8 = oe8_pool.tile([128, 2, 2, HP], FP8, tag="oe8",
                                        name=f"oe8_{S.b}_{j}")
                    # ones column (den accumulator) + pad cols, all 1.0;
                    # the h-halves copy below leaves cols HH:HP untouched
                    nc.gpsimd.memset(oe8[:, :, :, HH:HP], 1.0)
                    S.oe8_pairs[j] = oe8
                oe = oenat_pool.tile([128, H], BF16, tag="oe",
                                     name=f"oe_{S.b}_{2 * j + k}")
                oeT = oet_pool.tile([128, NH, 128], FP8, tag="oeT",
                                    name=f"oeT_{S.b}_{2 * j + k}")
                S.oe_tiles[2 * j + k] = oe
                S.oeT_tiles[2 * j + k] = oeT
                nc.vector.tensor_add(oe, sta, stb)
                transpose_tiles(oe, oeT)
                nc.vector.tensor_copy(S.oe8_pairs[j][:, k, :, 0:HH], oe)

            def tr_e(S, j):
                for k in range(2):
                    tr_e_k(S, j, k)

            def mm1(S, tci, i):
                psS = psS_pool.tile([128, TCHUNK], F32, tag="psS")
                for c2 in range(NH // 2):
                    nc.tensor.matmul(
                        psS,
                        S.oeT_tiles[i][:, 2 * c2:2 * c2 + 2, :],
                        S.odT_chunks[tci][:, 2 * c2:2 * c2 + 2, :],
                        start=(c2 == 0), stop=(c2 == NH // 2 - 1),
                        perf_mode=dr)
                P = p_pool.tile([128, TCHUNK], BF16, tag="P",
                                name=f"P_{S.b}_{tci}_{i}")
                # undo the SCALE^2 from the pre-scaled transposes
                nc.scalar.activation(P, psS, exp,
                                     scale=1.0 / (SCALE * SCALE))
                # d = DSCALE * (p - 1): fp8-friendly residual for mm2
                if i % 2 == 0:
                    d8 = d8_pool.tile([128, 2, TCHUNK], FP8, tag="d8",
                                      name=f"d8_{S.b}_{tci}_{i // 2}")
                    S.d8_pairs[tci][i // 2] = d8
                eng = nc.vector if i % 2 == 0 else nc.gpsimd
                eng.tensor_scalar(S.d8_pairs[tci][i // 2][:, i % 2, :],
                                  P, -1.0, DSCALE,
                                  mybir.AluOpType.add,
                                  mybir.AluOpType.mult)

            def colsum(S):
                # cs[h] = DSCALE * sum_s oe[s, h] (bf16 oe, exact part of
                # the p = 1 + d decomposition); packed as two halves of
                # HP cols each, with cols HH:HP = DSCALE*SL (den seed)
                pcs = ptr_pool.tile([1, 2, HH], F32, tag="ptr")
                for i in range(NS):
                    nc.tensor.matmul(pcs, ones, S.oe_tiles[i],
                                     start=(i == 0), stop=(i == NS - 1))
                cs2 = small_pool.tile([1, 2, HP], BF16, tag="cs", bufs=2)
                nc.vector.memset(cs2[:, :, HH:HP], float(DSCALE * SL))
                nc.vector.tensor_scalar(cs2[:, :, 0:HH], pcs, DSCALE, None,
                                        mybir.AluOpType.mult)
                S.cs2 = cs2

            def mm2(S, tci, feed=None):
                # two half-width accumulation groups per t-tile; the den
                # rides along as column HH of each (ones column in oe8).
                # The a-group completes before the b-group starts so
                # recip(psA) overlaps the b stream.  feed: iterator of
                # next-batch prep bundles, one consumed per t-tile
                # boundary (between accumulation groups, never inside).
                for tt in range(TPC):
                    if feed is not None:
                        bundle = next(feed, None)
                        if bundle is not None:
                            for op in bundle:
                                op()
                    psA = psC_pool.tile([128, HP], F32, tag="psC")
                    psB = psC_pool.tile([128, HP], F32, tag="psC")
                    nc.tensor.matmul(psA, onesK1, S.cs2[:, 0, :],
                                     start=True, stop=False)
                    nc.tensor.matmul(psB, onesK1, S.cs2[:, 1, :],
                                     start=True, stop=False)
                    for j in range(NS // 2):
                        nc.tensor.matmul(psA,
                                         S.d8_pairs[tci][j][:, :,
                                             tt * 128:(tt + 1) * 128],
                                         S.oe8_pairs[j][:, :, 0, :],
                                         start=False,
                                         stop=(j == NS // 2 - 1),
                                         perf_mode=dr)
                    rc = small_pool.tile([128, 1], F32, tag="rc")
                    nc.vector.reciprocal(rc, psA[:, HH:HH + 1])
                    for j in range(NS // 2):
                        nc.tensor.matmul(psB,
                                         S.d8_pairs[tci][j][:, :,
                                             tt * 128:(tt + 1) * 128],
                                         S.oe8_pairs[j][:, :, 1, :],
                                         start=False,
                                         stop=(j == NS // 2 - 1),
                                         perf_mode=dr)
                    ob = osb_pool.tile([128, H], F32, tag="ob")
                    nc.vector.tensor_scalar(ob[:, 0:HH], psA[:, 0:HH],
                                            rc, None,
                                            mybir.AluOpType.mult)
                    # B-half normalize on ScalarE (idle in the tail phase):
                    # splits the PSUM-bank-free path across two engines
                    nc.scalar.activation(ob[:, HH:H], psB[:, 0:HH],
                                         mybir.ActivationFunctionType.Copy,
                                         scale=rc)
                    t0 = tci * TCHUNK + tt * 128
                    nc.sync.dma_start(out[t0:t0 + 128, S.b, :], ob)

            def head_bundles(S, first=False):
                """A batch's full prep (loads, transposes, mm1, colsum) as
                16 bundles - one per mm2 t-tile boundary of the PREVIOUS
                batch (or run flat for the first batch).  DMAs lead their
                transposes by >=2 bundles so fed ops rarely wait at the
                head of an engine queue, and mm1 ops are woven with per-k
                e-transpose halves so the PE has transpose work while the
                Scalar exp chain (700ns per pair vs 430ns of matmuls)
                drains the psS pipeline."""
                def dD(ci):
                    return lambda: dma_d(S, ci)

                def tD(ci):
                    return lambda: tr_d(S, ci)

                def dE(j):
                    return lambda: dma_e(S, j)

                def tE(j):
                    return lambda: tr_e(S, j)

                def tek(j, k):
                    return lambda: tr_e_k(S, j, k)

                def mi(s, t):
                    return lambda: mm1(S, t, s)

                def m1(s):
                    return [mi(s, t) for t in range(NTC)]

                def weave(j, s):
                    return [tek(j, 0), mi(s, 0), mi(s, 1),
                            tek(j, 1), mi(s, 2), mi(s, 3)]

                if first:
                    # dD(0)/dD(1)/dE(0) were already issued pre-warmup so
                    # the first chunk lands right as the warmup ends
                    head = [[lambda: tr_d_split(S, 0)],
                            [tD(1), dE(1), dD(2)],
                            [tE(0), dE(2), dD(3)],
                            [dE(3)],
                            [tD(2), dE(4)]]
                else:
                    head = [[dD(0), dD(1)], [dE(0)],
                            [tD(0), dE(1), dD(2)],
                            [tD(1), dE(2), dD(3)],
                            [tE(0), dE(3)],
                            [tD(2), dE(4)]]
                return head + [
                    [tD(3), dE(5)] + m1(0),
                    weave(1, 1) + [dE(6)] + m1(2),
                    weave(2, 3) + [dE(7)] + m1(4),
                    weave(3, 5) + m1(6),
                    weave(4, 7) + m1(8),
                    weave(5, 9) + m1(10),
                    weave(6, 11) + m1(12),
                    weave(7, 13),
                    m1(14) + m1(15),
                    [lambda: colsum(S)],
                ]

            def prep_b0(S):
                """Batch-0 prep, flat emission in data-arrival order:
                d-chunk0 + e-tile0 ride the two HWDGE queues and land by
                ~17us, so chunk-0 mm1 starts immediately; later chunks
                backfill as their SWDGE loads land.  SWDGE issues (gE/gD)
                are placed so each stream is requested ~2 consumers
                ahead."""
                def gE(j):
                    return lambda: dma_e(S, j)

                def gD(ci):
                    return lambda: dma_d(S, ci)

                def tD(ci):
                    return lambda: tr_d(S, ci)

                def tek(j, k):
                    return lambda: tr_e_k(S, j, k)

                def mi(s, t):
                    return lambda: mm1(S, t, s)

                return [
                    lambda: tr_d_split(S, 0),
                    tek(0, 0), mi(0, 0), tek(0, 1), mi(1, 0),
                    gE(1),
                    tD(1), mi(0, 1), mi(1, 1),
                    tek(1, 0), mi(2, 0), mi(2, 1),
                    tek(1, 1), mi(3, 0), mi(3, 1),
                    gE(2), gD(2),
                    tek(2, 0), mi(4, 0), mi(4, 1),
                    tek(2, 1), mi(5, 0), mi(5, 1),
                    tD(2), mi(0, 2), mi(1, 2), mi(2, 2), mi(3, 2),
                    mi(4, 2), mi(5, 2),
                    gE(3), gD(3),
                    tek(3, 0), mi(6, 0), mi(6, 1), mi(6, 2),
                    tek(3, 1), mi(7, 0), mi(7, 1), mi(7, 2),
                    gE(4),
                    tD(3), mi(0, 3), mi(1, 3), mi(2, 3), mi(3, 3),
                    tek(4, 0), mi(8, 0), mi(8, 1),
                    tek(4, 1), mi(9, 0), mi(9, 1),
                    gE(5),
                    mi(4, 3), mi(5, 3), mi(6, 3), mi(7, 3),
                    mi(8, 2), mi(8, 3), mi(9, 2), mi(9, 3),
                    tek(5, 0), mi(10, 0), mi(10, 1),
                    tek(5, 1), mi(11, 0), mi(11, 1),
                    gE(6),
                    mi(10, 2), mi(10, 3), mi(11, 2), mi(11, 3),
                    tek(6, 0), mi(12, 0), mi(12, 1),
                    tek(6, 1), mi(13, 0), mi(13, 1),
                    gE(7),
                    mi(12, 2), mi(12, 3), mi(13, 2), mi(13, 3),
                    tek(7, 0), mi(14, 0), mi(14, 1),
                    tek(7, 1), mi(15, 0), mi(15, 1),
                    mi(14, 2), mi(14, 3), mi(15, 2), mi(15, 3),
                    lambda: colsum(S),
                ]

            S0 = BatchState(0)
            # critical first loads on the two HWDGE queues (they start
            # transferring immediately; SWDGE ramps for ~10us): e-tile0
            # f32 on the Scalar queue, d-chunk0 f32 on the Sync queue
            dma_e_split(S0, 0)
            dma_d(S0, 0, split=True)
            dma_d(S0, 1)     # first SWDGE stream: ramps during the warmup

            # HAM warmup: un-throttle the PE clock before the load phase.
            warm = consts.tile([128, TCHUNK], BF16, tag="warm")
            nc.vector.memset(warm, 0.25)
            wt = ptr_pool.tile([128, TCHUNK], F32, tag="ptr")
            for _ in range(10):
                nc.tensor.matmul(wt, warm[:, 0:128], warm,
                                 start=True, stop=True)

            # SWDGE holdback: this GpSimd op waits for the critical HWDGE
            # loads, so the SWDGE issues queued behind it can't start
            # transfers that would steal HBM bandwidth from them.
            blk = small_pool.tile([128, 1], F32, tag="blk", bufs=1)
            nc.gpsimd.tensor_scalar(blk, S0.st[0][1][:, 0:1], 1.0, None,
                                    mybir.AluOpType.mult)

            for op in prep_b0(S0):
                op()
            assert all(t is not None for t in S0.oeT_tiles)
            assert all(S0.d8_pairs[c][j] is not None
                       for c in range(NTC) for j in range(NS // 2))

            S1 = BatchState(1)
            feed = iter(head_bundles(S1))
            for tci in range(NTC):
                mm2(S0, tci, feed=feed)
            for bundle in feed:
                for op in bundle:
                    op()
            for tci in range(NTC):
                mm2(S1, tci)

    nc.compile()
    return nc


_nc = None
last_result = None
_IDENT = (np.eye(128) * SCALE).astype(ml_dtypes.bfloat16)


def kernel(in_e=None, out_e=None, out_d=None, _trace=False, **_unused):
    global _nc, last_result
    if _nc is None:
        _nc = build()
    out_e = np.asarray(out_e, dtype=np.float32)
    out_d = np.asarray(out_d, dtype=np.float32)
    in_maps = []
    for c in range(NCORES):
        sl = slice(c * BPC, (c + 1) * BPC)
        in_maps.append({
            "out_e": np.ascontiguousarray(out_e[:, sl, :]),
            "out_d": np.ascontiguousarray(out_d[:, sl, :]),
            "ident": _IDENT,
        })
    last_result = run_bass_kernel_spmd(_nc, in_maps,
                                       core_ids=list(range(NCORES)),
                                       trace=_trace)
    return np.concatenate(
        [np.asarray(last_result.results[c]["out"]) for c in range(NCORES)],
        axis=1).astype(np.float32)

